# revision 8
# baseline (speedup 1.0000x reference)
"""BiCutLoss Trainium2 kernel (nn_BiCutLoss_52312701665760).

Reference computation (per batch row i of output[B, L, 2], labels[B, L]):
  temp = argmax(output, -1)            # 1 iff out1 > out0
  cut  = L if all(temp == 1) else (index of last 0 in temp)
  mask = arange(L) < cut
  r1   = where(labels == 1, -3.6/log2(j+2), 0.065)
  loss = sum(out1 * mask * r1) / B

Kernel formulation (exactly equivalent):
  d[j] = out0[j] - out1[j]                       # temp[j]==0  <=>  d[j] >= 0
  M[j] = max(d[j:], -1)  (reverse cummax; M[L] = -1 pad)
  thr  = 0 if M[0] >= 0 else -BIG                # all-ones row => mask all 1
  mask[j] = (M[j+1] >= thr)
  r1   = C + lab*preD   with C = 0.065, preD[j] = -3.6/log2(j+2) - C
  S_i  = sum_j mask*t1          A_i = sum_j mask*t1*lab*preD
  loss_i = C*S_i + A_i

Sharding: pure data parallel - B=4096 rows split as 512 rows x 8 cores; each
core computes per-row partials [128,1] (4 row-tiles of 128 partitions), host
sums and divides by B.

Engine assignment per [128, 4096] row-tile (HBM floor ~70us/iter per core is
the target bound; keep every engine below it):
  SP  (HWDGE): ot DMA (4.19 MB interleaved f32)
  ACT (HWDGE): lab DMA (2.1 MB raw int32)  -- off Pool's SWDGE path
  ACT: lab_b = Copy(lab_i32) -> f16 cast; sink Copy(w) accum -> A_k
  Pool: d = t0 - t1 (strided f32 in, f16 out)  -- its only big op
  DVE : z = lab_b*preD_b (TT f16 2x), reverse scan max (1x), thr (tiny),
        w = (M[1:] >= thr)*t1 (STT 1x, accum -> S_k), w *= z (TT f16 2x)
  All DVE 2x ops are 2x_1P (single-port) so GPSIMD SBUF access is not
  locked out.
"""

import os
from contextlib import ExitStack

import numpy as np

B, L = 4096, 4096
N_CORES = 8
ROWS_PER_CORE = B // N_CORES          # 512
P = 128                               # partitions per tile
TILES = ROWS_PER_CORE // P            # 4
C_CONST = 0.65 * 0.1                  # 0.065
BIG = 1e30

_CACHE = {}


def _build_nc(repeat: int = 1):
    import concourse.mybir as mybir
    import concourse.tile as tile
    from concourse import bacc

    f32 = mybir.dt.float32
    f16 = mybir.dt.float16
    i32 = mybir.dt.int32
    Op = mybir.AluOpType
    Act = mybir.ActivationFunctionType

    # Bacc (not raw Bass): its compile() runs generate_event_semaphores,
    # which splits multi-sem waits into standalone EventSemaphore
    # instructions (HW allows at most 1 wait per compute instruction).
    nc = bacc.Bacc("TRN2", target_bir_lowering=False, debug=False)

    out_d = nc.dram_tensor("out", [ROWS_PER_CORE, L * 2], f32, kind="ExternalInput")
    lab_d = nc.dram_tensor("lab", [ROWS_PER_CORE, L], i32, kind="ExternalInput")
    pre_d = nc.dram_tensor("pre", [P, L], f32, kind="ExternalInput")
    res_d = nc.dram_tensor("res", [P, 1], f32, kind="ExternalOutput")

    out_t = out_d[:].rearrange("(n p) m -> n p m", p=P)   # [4, 128, 8192]
    lab_t = lab_d[:].rearrange("(n p) m -> n p m", p=P)   # [4, 128, 4096]

    with tile.TileContext(nc) as tc, ExitStack() as ctx:
        io_pool = ctx.enter_context(tc.tile_pool(name="io", bufs=2))
        pre_pool = ctx.enter_context(tc.tile_pool(name="pre", bufs=1))
        d_pool = ctx.enter_context(tc.tile_pool(name="d", bufs=2))
        m_pool = ctx.enter_context(tc.tile_pool(name="m", bufs=2))
        w_pool = ctx.enter_context(tc.tile_pool(name="w", bufs=2))
        z_pool = ctx.enter_context(tc.tile_pool(name="z", bufs=2))
        acc_pool = ctx.enter_context(tc.tile_pool(name="acc", bufs=2))

        # preamble (outside the repeat loop; excluded from per-iter time):
        # preD as bf16 via SWDGE cast DMA, one-time.
        pre_b = pre_pool.tile([P, L], f16)
        nc.gpsimd.dma_start(pre_b[:], pre_d[:])

        # labels ride the second HWDGE ring (ACT issues) but are issued one
        # tile AHEAD of use, so the issue instruction sits in front of the
        # convert/sink pair in ACT's in-order queue and the transfer overlaps
        # the previous tile's compute. SP's ring then carries only ot.
        flat = [(r, k) for r in range(repeat) for k in range(TILES)]
        lt_cur = io_pool.tile([P, L], i32, tag="lt")
        nc.scalar.dma_start(lt_cur[:], lab_t[0])
        acc_S = acc_A = None
        for n, (_r, k) in enumerate(flat):
            if k == 0:
                acc_S = acc_pool.tile([P, TILES], f32, tag="accS")
                acc_A = acc_pool.tile([P, TILES], f32, tag="accA")
            if True:
                ot = io_pool.tile([P, L * 2], f32, tag="ot")
                nc.sync.dma_start(ot[:], out_t[k])
                lt = lt_cur
                if n + 1 < len(flat):
                    lt_cur = io_pool.tile([P, L], i32, tag="lt")
                    nc.scalar.dma_start(lt_cur[:], lab_t[flat[n + 1][1]])
                # ACT: int32 -> f16 cast on-engine.
                lab_b = z_pool.tile([P, L], f16, tag="labb")
                nc.scalar.activation(lab_b[:], lt[:], Act.Copy)

                x3 = ot[:].rearrange("p (l c) -> p l c", c=2)
                t0 = x3[:, :, 0]
                t1 = x3[:, :, 1]

                # Pool's only heavy op: d = t0 - t1 (f16 out).
                d = d_pool.tile([P, L], f16)
                nc.gpsimd.tensor_tensor(d[:], t0, t1, Op.subtract)

                # DVE scan: M[j] = max(d[j:], -1), M[L] = -1 pad (f16).
                # Ordered before z so ACT's sink(k-1)+convert(k) hide under
                # the scan+STT window instead of stalling DVE's queue head.
                M = m_pool.tile([P, L + 1], f16)
                nc.vector.memset(M[:, L:L + 1], -1.0)
                nc.vector.tensor_tensor_scan(
                    M[:, 0:L][:, ::-1], d[:, ::-1], d[:, ::-1], -1.0,
                    Op.max, Op.max,
                )

                # tiny (DVE): thr = 0 if M[0] >= 0 else -BIG, one fused TS:
                # (M0 < 0) * -BIG
                thr = acc_pool.tile([P, 1], f32, tag="thr")
                nc.vector.tensor_scalar(
                    thr[:], M[:, 0:1], 0.0, -BIG, Op.is_lt, Op.mult
                )

                # DVE: w = (M[j+1] >= thr) * t1 (f16 out), S_k = sum(w).
                w = w_pool.tile([P, L], f16)
                nc.vector.scalar_tensor_tensor(
                    w[:], M[:, 1:L + 1], thr[:], t1,
                    Op.is_ge, Op.mult,
                    accum_out=acc_S[:, k:k + 1],
                )

                # DVE: z = lab_b * preD (TT f16, 2x), late on purpose.
                z = z_pool.tile([P, L], f16, tag="z")
                nc.vector.tensor_tensor(z[:], lab_b[:], pre_b[:], Op.mult)

                # DVE: w *= z (TT f16, 2x, in-place).
                nc.vector.tensor_tensor(w[:], w[:], z[:], Op.mult)

                # ACT: sink copy with accumulator -> A_k = sum(w*z).
                sink = w_pool.tile([P, L], f16, tag="sink")
                nc.scalar.activation(
                    sink[:], w[:], Act.Copy,
                    accum_out=acc_A[:, k:k + 1],
                )

            if k == TILES - 1:
                # tail: loss_i = C*sum_k S_k + sum_k A_k
                t4 = acc_pool.tile([P, TILES], f32, tag="t4")
                nc.vector.tensor_scalar(t4[:], acc_S[:], C_CONST, None, Op.mult)
                nc.vector.tensor_tensor(t4[:], t4[:], acc_A[:], Op.add)
                loss_t = acc_pool.tile([P, 1], f32, tag="loss")
                nc.vector.reduce_sum(loss_t[:], t4[:], axis=mybir.AxisListType.X)

        nc.sync.dma_start(res_d[:], loss_t[:])

    nc.compile()
    return nc


def _pre_tile() -> np.ndarray:
    j = np.arange(L, dtype=np.float64)
    pre2 = (-3.6 / np.log2(j + 2.0) - C_CONST).astype(np.float32)
    return np.ascontiguousarray(np.tile(pre2[None, :], (P, 1)))


def _get_nc(repeat: int = 1):
    key = repeat
    if key not in _CACHE:
        _CACHE[key] = _build_nc(repeat=repeat)
    return _CACHE[key]


def make_in_maps(output: np.ndarray, labels: np.ndarray):
    pre = _pre_tile()
    in_maps = []
    for c in range(N_CORES):
        sl = slice(c * ROWS_PER_CORE, (c + 1) * ROWS_PER_CORE)
        in_maps.append({
            "out": np.ascontiguousarray(output[sl]).reshape(ROWS_PER_CORE, L * 2),
            "lab": np.ascontiguousarray(labels[sl]),
            "pre": pre,
        })
    return in_maps


def kernel(output: np.ndarray, labels: np.ndarray) -> np.ndarray:
    from concourse.bass_utils import run_bass_kernel_spmd

    nc = _get_nc(repeat=1)
    in_maps = make_in_maps(output, labels)
    r = run_bass_kernel_spmd(nc, in_maps, core_ids=list(range(N_CORES)))
    total = 0.0
    for res in r.results:
        total += float(res["res"].astype(np.float64).sum())
    return np.float32(total / B)


if __name__ == "__main__":
    # quick standalone run (full inputs, random)
    rng = np.random.default_rng(0)
    out = rng.standard_normal((B, L, 2)).astype(np.float32)
    lab = rng.integers(0, 2, size=(B, L)).astype(np.int32)
    print("loss:", kernel(out, lab))


# revision 9
# speedup vs baseline: 1.1269x; 1.1269x over previous
"""BiCutLoss Trainium2 kernel (nn_BiCutLoss_52312701665760).

Reference computation (per batch row i of output[B, L, 2], labels[B, L]):
  temp = argmax(output, -1)            # 1 iff out1 > out0
  cut  = L if all(temp == 1) else (index of last 0 in temp)
  mask = arange(L) < cut
  r1   = where(labels == 1, -3.6/log2(j+2), 0.065)
  loss = sum(out1 * mask * r1) / B

Kernel formulation (exactly equivalent):
  d[j] = out0[j] - out1[j]                       # temp[j]==0  <=>  d[j] >= 0
  M[j] = max(d[j:], -1)  (reverse cummax; M[L] = -1 pad)
  thr  = 0 if M[0] >= 0 else -BIG                # all-ones row => mask all 1
  mask[j] = (M[j+1] >= thr)
  r1   = C + lab*preD   with C = 0.065, preD[j] = -3.6/log2(j+2) - C
  S_i  = sum_j mask*t1          A_i = sum_j mask*t1*lab*preD
  loss_i = C*S_i + A_i

Sharding: pure data parallel - B=4096 rows split as 512 rows x 8 cores; each
core computes per-row partials [128,1] (4 row-tiles of 128 partitions), host
sums and divides by B.

Engine assignment per [128, 4096] row-tile (HBM floor ~70us/iter per core is
the target bound; keep every engine below it):
  SP  (HWDGE): ot DMA (4.19 MB interleaved f32)
  ACT (HWDGE): lab DMA (2.1 MB raw int32)  -- off Pool's SWDGE path
  ACT: lab_b = Copy(lab_i32) -> f16 cast; sink Copy(w) accum -> A_k
  Pool: d = t0 - t1 (strided f32 in, f16 out)  -- its only big op
  DVE : z = lab_b*preD_b (TT f16 2x), reverse scan max (1x), thr (tiny),
        w = (M[1:] >= thr)*t1 (STT 1x, accum -> S_k), w *= z (TT f16 2x)
  All DVE 2x ops are 2x_1P (single-port) so GPSIMD SBUF access is not
  locked out.
"""

import os
from contextlib import ExitStack

import numpy as np

B, L = 4096, 4096
N_CORES = 8
ROWS_PER_CORE = B // N_CORES          # 512
P = 128                               # partitions per tile
TILES = ROWS_PER_CORE // P            # 4
C_CONST = 0.65 * 0.1                  # 0.065
BIG = 1e30

_CACHE = {}


def _build_nc(repeat: int = 1):
    import concourse.mybir as mybir
    import concourse.tile as tile
    from concourse import bacc

    f32 = mybir.dt.float32
    f16 = mybir.dt.float16
    i32 = mybir.dt.int32
    Op = mybir.AluOpType
    Act = mybir.ActivationFunctionType

    # Bacc (not raw Bass): its compile() runs generate_event_semaphores,
    # which splits multi-sem waits into standalone EventSemaphore
    # instructions (HW allows at most 1 wait per compute instruction).
    nc = bacc.Bacc("TRN2", target_bir_lowering=False, debug=False)

    out_d = nc.dram_tensor("out", [ROWS_PER_CORE, L * 2], f32, kind="ExternalInput")
    lab_d = nc.dram_tensor("lab", [ROWS_PER_CORE, L], i32, kind="ExternalInput")
    pre_d = nc.dram_tensor("pre", [P, L], f32, kind="ExternalInput")
    res_d = nc.dram_tensor("res", [P, 1], f32, kind="ExternalOutput")

    out_t = out_d[:].rearrange("(n p) m -> n p m", p=P)   # [4, 128, 8192]
    lab_t = lab_d[:].rearrange("(n p) m -> n p m", p=P)   # [4, 128, 4096]

    with tile.TileContext(nc) as tc, ExitStack() as ctx:
        io_pool = ctx.enter_context(tc.tile_pool(name="io", bufs=2))
        pre_pool = ctx.enter_context(tc.tile_pool(name="pre", bufs=1))
        d_pool = ctx.enter_context(tc.tile_pool(name="d", bufs=2))
        m_pool = ctx.enter_context(tc.tile_pool(name="m", bufs=2))
        w_pool = ctx.enter_context(tc.tile_pool(name="w", bufs=2))
        z_pool = ctx.enter_context(tc.tile_pool(name="z", bufs=2))
        acc_pool = ctx.enter_context(tc.tile_pool(name="acc", bufs=2))

        # preamble (outside the repeat loop; excluded from per-iter time):
        # preD as bf16 via SWDGE cast DMA, one-time.
        pre_b = pre_pool.tile([P, L], f16)
        nc.gpsimd.dma_start(pre_b[:], pre_d[:])

        for _r in range(repeat):
            acc_S = acc_pool.tile([P, TILES], f32, tag="accS")
            acc_A = acc_pool.tile([P, TILES], f32, tag="accA")
            for k in range(TILES):
                ot = io_pool.tile([P, L * 2], f32, tag="ot")
                nc.sync.dma_start(ot[:], out_t[k])
                lt = io_pool.tile([P, L], i32, tag="lt")
                nc.sync.dma_start(lt[:], lab_t[k])
                # ACT: int32 -> f16 cast on-engine.
                lab_b = z_pool.tile([P, L], f16, tag="labb")
                nc.scalar.activation(lab_b[:], lt[:], Act.Copy)

                x3 = ot[:].rearrange("p (l c) -> p l c", c=2)
                t0 = x3[:, :, 0]
                t1 = x3[:, :, 1]

                # Pool's only heavy op: d = t0 - t1 (f16 out).
                d = d_pool.tile([P, L], f16)
                nc.gpsimd.tensor_tensor(d[:], t0, t1, Op.subtract)

                # DVE scan: M[j] = max(d[j:], -1), M[L] = -1 pad (f16).
                # Ordered before z so ACT's sink(k-1)+convert(k) hide under
                # the scan+STT window instead of stalling DVE's queue head.
                M = m_pool.tile([P, L + 1], f16)
                nc.vector.memset(M[:, L:L + 1], -1.0)
                nc.vector.tensor_tensor_scan(
                    M[:, 0:L][:, ::-1], d[:, ::-1], d[:, ::-1], -1.0,
                    Op.max, Op.max,
                )

                # tiny (DVE): thr = 0 if M[0] >= 0 else -BIG, one fused TS:
                # (M0 < 0) * -BIG
                thr = acc_pool.tile([P, 1], f32, tag="thr")
                nc.vector.tensor_scalar(
                    thr[:], M[:, 0:1], 0.0, -BIG, Op.is_lt, Op.mult
                )

                # DVE: w = (M[j+1] >= thr) * t1 (f16 out), S_k = sum(w).
                w = w_pool.tile([P, L], f16)
                nc.vector.scalar_tensor_tensor(
                    w[:], M[:, 1:L + 1], thr[:], t1,
                    Op.is_ge, Op.mult,
                    accum_out=acc_S[:, k:k + 1],
                )

                # DVE: z = lab_b * preD (TT f16, 2x), late on purpose.
                z = z_pool.tile([P, L], f16, tag="z")
                nc.vector.tensor_tensor(z[:], lab_b[:], pre_b[:], Op.mult)

                # DVE: w *= z (TT f16, 2x, in-place).
                nc.vector.tensor_tensor(w[:], w[:], z[:], Op.mult)

                # ACT: sink copy with accumulator -> A_k = sum(w*z).
                sink = w_pool.tile([P, L], f16, tag="sink")
                nc.scalar.activation(
                    sink[:], w[:], Act.Copy,
                    accum_out=acc_A[:, k:k + 1],
                )

            if k == TILES - 1:
                # tail: loss_i = C*sum_k S_k + sum_k A_k
                t4 = acc_pool.tile([P, TILES], f32, tag="t4")
                nc.vector.tensor_scalar(t4[:], acc_S[:], C_CONST, None, Op.mult)
                nc.vector.tensor_tensor(t4[:], t4[:], acc_A[:], Op.add)
                loss_t = acc_pool.tile([P, 1], f32, tag="loss")
                nc.vector.reduce_sum(loss_t[:], t4[:], axis=mybir.AxisListType.X)

        nc.sync.dma_start(res_d[:], loss_t[:])

    nc.compile()
    return nc


def _pre_tile() -> np.ndarray:
    j = np.arange(L, dtype=np.float64)
    pre2 = (-3.6 / np.log2(j + 2.0) - C_CONST).astype(np.float32)
    return np.ascontiguousarray(np.tile(pre2[None, :], (P, 1)))


def _get_nc(repeat: int = 1):
    key = repeat
    if key not in _CACHE:
        _CACHE[key] = _build_nc(repeat=repeat)
    return _CACHE[key]


def make_in_maps(output: np.ndarray, labels: np.ndarray):
    pre = _pre_tile()
    in_maps = []
    for c in range(N_CORES):
        sl = slice(c * ROWS_PER_CORE, (c + 1) * ROWS_PER_CORE)
        in_maps.append({
            "out": np.ascontiguousarray(output[sl]).reshape(ROWS_PER_CORE, L * 2),
            "lab": np.ascontiguousarray(labels[sl]),
            "pre": pre,
        })
    return in_maps


def kernel(output: np.ndarray, labels: np.ndarray) -> np.ndarray:
    from concourse.bass_utils import run_bass_kernel_spmd

    nc = _get_nc(repeat=1)
    in_maps = make_in_maps(output, labels)
    r = run_bass_kernel_spmd(nc, in_maps, core_ids=list(range(N_CORES)))
    total = 0.0
    for res in r.results:
        total += float(res["res"].astype(np.float64).sum())
    return np.float32(total / B)


if __name__ == "__main__":
    # quick standalone run (full inputs, random)
    rng = np.random.default_rng(0)
    out = rng.standard_normal((B, L, 2)).astype(np.float32)
    lab = rng.integers(0, 2, size=(B, L)).astype(np.int32)
    print("loss:", kernel(out, lab))
